# revision 1
# baseline (speedup 1.0000x reference)
"""Trainium2 Bass kernel: 21 depthwise Gaussian blurs + channel concat.

Problem: x (8, 3, 512, 512) f32 -> out (8, 66, 512, 512) f32 where
out = concat([x, blur_0(x), ..., blur_20(x)], axis=1) and blur_i is a
depthwise 2D Gaussian conv (reflect padding, kernel sizes 3..21).

Strategy (pure data parallel, 1 image per core across 8 cores):
  Each Gaussian is separable: conv2d(outer(g,g)) = conv_H(g) then conv_W(g).
  Each 1D conv (with reflect pad folded in) is a banded 512x512 matrix M.
  Per channel-image X (512x512):   Y_i = M_i @ X @ M_i^T
  Pass 1 computes Z^T = (M X)^T via out = lhsT.T @ rhs with lhsT = X-block
  (stationary) and rhs = band slabs of M^T (moving, ~130-148 cols per
  128-row block).  Pass 2 repeats the same structure on Z^T, which
  transposes back, yielding Y in natural [h, w] layout.  Both passes
  stream the same bf16 band slabs; PSUM accumulates in fp32 using
  per-element has_written semantics across overlapping column ranges.
"""

import numpy as np
import ml_dtypes

B, C, H, W = 8, 3, 512, 512
N = 512
P = 128
NBLK = N // P  # 4

NUM_KERNELS = 21
MAX_KSIZE = 21
INIT_KSIZE = 3
_INCREMENT = (MAX_KSIZE - INIT_KSIZE) / (NUM_KERNELS - 2)
KSIZES = [
    min(MAX_KSIZE, int(INIT_KSIZE + i * _INCREMENT // 2 * 2))
    for i in range(NUM_KERNELS)
]
SIGMAS = np.linspace(0.5, 1.0, NUM_KERNELS)

TRACE = False  # set True (from a driver) to capture an NTFF profile
MERGE_LDW = False  # fold standalone LDWEIGHTS back into self-loading matmuls
LDW_OPT = False  # pass --enable-ldw-opt=true to walrus (requires MERGE_LDW)
LAST_RESULTS = {}  # driver-inspectable: exec_time_ns etc.


def _gauss1d(k, sigma):
    # Matches reference _gauss_kernel numerics: float32 arange, float64 sigma
    # promotes the math to float64; normalized to sum 1.
    x = np.arange(k, dtype=np.float32)
    g = np.exp(-((x - k // 2) ** 2) / (2.0 * sigma**2))
    return g / g.sum()


def _conv_matrix(g, n=N):
    """Banded matrix M (float64) s.t. y = M @ x computes the reflect-padded
    1D convolution with taps g."""
    k = len(g)
    p = (k - 1) // 2
    M = np.zeros((n, n), np.float64)
    for r in range(n):
        for t in range(k):
            c = r + t - p
            if c < 0:
                c = -c
            elif c >= n:
                c = 2 * (n - 1) - c
            M[r, c] += g[t]
    return M


def _slab_geometry():
    """Per (kernel, block) slab column ranges in M^T, plus pack offsets."""
    geo = []  # [i][b] = (clo, chi, off)
    off = 0
    for i in range(NUM_KERNELS):
        p = (KSIZES[i] - 1) // 2
        row = []
        for b in range(NBLK):
            clo = max(0, P * b - p)
            chi = min(N, P * b + P + p)
            row.append((clo, chi, off))
            off += chi - clo
        geo.append(row)
    return geo, off


def _build_wpack():
    geo, totalw = _slab_geometry()
    wpack = np.zeros((P, totalw), ml_dtypes.bfloat16)
    for i in range(NUM_KERNELS):
        MT = _conv_matrix(_gauss1d(KSIZES[i], SIGMAS[i])).T
        for b in range(NBLK):
            clo, chi, off = geo[i][b]
            wpack[:, off : off + (chi - clo)] = MT[P * b : P * b + P, clo:chi].astype(
                ml_dtypes.bfloat16
            )
    return geo, totalw, wpack


_GEO, _TOTALW, _WPACK = None, None, None
_NC = None


def _consts():
    global _GEO, _TOTALW, _WPACK
    if _WPACK is None:
        _GEO, _TOTALW, _WPACK = _build_wpack()
    return _GEO, _TOTALW, _WPACK


def _build_nc():
    import concourse.bacc as bacc
    import concourse.mybir as mybir
    from concourse.tile import TileContext

    geo, totalw, _ = _consts()
    bf16 = mybir.dt.bfloat16
    f32 = mybir.dt.float32

    nc = bacc.Bacc("TRN2", target_bir_lowering=False)
    x = nc.dram_tensor("x", [C, N, N], f32, kind="ExternalInput")
    w = nc.dram_tensor("w", [P, totalw], bf16, kind="ExternalInput")
    y = nc.dram_tensor("y", [C * (NUM_KERNELS + 1), N, N], f32, kind="ExternalOutput")

    ncopy = 0  # alternate PSUM-evacuation copies between DVE and ACT

    with TileContext(nc) as tc:
        with (
            tc.tile_pool(name="wsb", bufs=1) as wpool,
            tc.tile_pool(name="xf", bufs=12) as xfpool,
            tc.tile_pool(name="xsb", bufs=8) as xpool,
            tc.tile_pool(name="zt", bufs=6) as ztpool,
            tc.tile_pool(name="yo", bufs=4) as ypool,
            tc.tile_pool(name="ps1", bufs=2, space="PSUM") as ps1,
            tc.tile_pool(name="ps2", bufs=2, space="PSUM") as ps2,
        ):
            # x loads first: fp32 tiles (also serve the passthrough write),
            # then bf16 casts on the otherwise-idle GpSimd engine.
            xf = {}
            xbf = {}
            for ci in range(C):
                for j in range(NBLK):
                    t = xfpool.tile([P, N], f32, tag="xf")
                    nc.sync.dma_start(t[:], x[ci, P * j : P * j + P, :])
                    xf[ci, j] = t
            for ci in range(C):
                for j in range(NBLK):
                    t = xpool.tile([P, N], bf16, tag="x")
                    nc.gpsimd.tensor_copy(t[:], xf[ci, j][:])
                    xbf[ci, j] = t

            # band slabs, chunked so early kernels start before the full load
            wsb = wpool.tile([P, totalw], bf16)
            bounds = [geo[i][0][2] for i in range(0, NUM_KERNELS, 3)] + [totalw]
            for a, b in zip(bounds[:-1], bounds[1:]):
                nc.sync.dma_start(wsb[:, a:b], w[:, a:b])

            for ci in range(C):
                xtiles = [xbf[ci, j] for j in range(NBLK)]
                for i in range(NUM_KERNELS):
                    # ---- pass 1: Z^T[wb] = sum_j X[j,wb]^T @ slab(i,j) ----
                    # two wb blocks share one 2-bank PSUM tile -> one big copy
                    zt = []
                    for wb2 in range(NBLK // 2):
                        psz = ps1.tile([P, 2 * N], f32, tag="psz")
                        for half in range(2):
                            wb = 2 * wb2 + half
                            for j in range(NBLK):
                                clo, chi, off = geo[i][j]
                                nc.tensor.matmul(
                                    psz[:, half * N + clo : half * N + chi],
                                    xtiles[j][:, P * wb : P * wb + P],
                                    wsb[:, off : off + (chi - clo)],
                                    start=(j == 0),
                                    stop=(j == NBLK - 1),
                                )
                        zt2 = ztpool.tile([P, 2 * N], bf16, tag="zt")
                        if ncopy % 2 == 0:
                            nc.vector.tensor_copy(zt2[:], psz[:])
                        else:
                            nc.scalar.copy(zt2[:], psz[:])
                        ncopy += 1
                        zt.append(zt2)

                    def ztap(wb, hb):
                        # Z^T[wb] block, columns for h-block hb
                        return zt[wb // 2][:, (wb % 2) * N + P * hb : (wb % 2) * N + P * hb + P]

                    # ---- pass 2: Y[hb] = sum_wb Z^T[wb,hb]^T @ slab(i,wb) ----
                    cout = C * (i + 1) + ci
                    for hb2 in range(NBLK // 2):
                        psy = ps2.tile([P, 2 * N], f32, tag="psy")
                        for half in range(2):
                            hb = 2 * hb2 + half
                            for wb in range(NBLK):
                                clo, chi, off = geo[i][wb]
                                nc.tensor.matmul(
                                    psy[:, half * N + clo : half * N + chi],
                                    ztap(wb, hb),
                                    wsb[:, off : off + (chi - clo)],
                                    start=(wb == 0),
                                    stop=(wb == NBLK - 1),
                                )
                        yo = ypool.tile([P, 2 * N], f32, tag="yo")
                        if ncopy % 2 == 0:
                            nc.vector.tensor_copy(yo[:], psy[:])
                        else:
                            nc.scalar.copy(yo[:], psy[:])
                        ncopy += 1
                        nc.sync.dma_start(
                            y[cout, 2 * P * hb2 : 2 * P * hb2 + 2 * P, :].rearrange(
                                "(b p) w -> p b w", b=2
                            ),
                            yo[:].rearrange("p (b w) -> p b w", b=2),
                        )


            # passthrough: out channels 0..2 = x (from the fp32 SBUF tiles)
            for ci in range(C):
                for j in range(NBLK):
                    nc.sync.dma_start(y[ci, P * j : P * j + P, :], xf[ci, j][:])

    nc.finalize()
    if MERGE_LDW:
        _merge_ldweights(nc)
    return nc


def _merge_ldweights(nc):
    """Fold the Tile-emitted standalone InstLdweights back into their
    InstMatmult (self-loading form) so walrus's LDW optimization (fast
    weight load) can apply.  LDWs carrying sync waits are replaced by an
    InstEventSemaphore stub at the same position to preserve ordering."""
    import concourse.mybir as mybir

    ev = 0
    for blk in nc.m.functions[0].blocks:
        insts = blk.instructions
        new = []
        changed = False
        for ins in insts:
            tn = type(ins).__name__
            if tn == "InstLdweights":
                changed = True
                si = ins.sync_info
                if si is not None and (si.on_wait or si.on_update):
                    e = mybir.InstEventSemaphore(
                        name=f"ldw_ev_{ev}", ins=[], outs=[]
                    )
                    ev += 1
                    e.engine = ins.engine
                    e.sync_info = si
                    new.append(e)
                continue
            if tn == "InstMatmult":
                ins.ldweights = True
            new.append(ins)
        if changed:
            del insts[:]
            insts.extend(new)


def _get_nc():
    global _NC
    if _NC is None:
        _NC = _build_nc()
    return _NC


def _install_trace_hook():
    """Best-effort NTFF profiling hook for axon (used when TRACE=True)."""
    import sys
    import types

    if "antenv.axon_hooks" in sys.modules:
        return
    m = types.ModuleType("antenv.axon_hooks")
    m._hook = None
    m.set_axon_ntff_profile_hook = lambda h: setattr(m, "_hook", h)
    m.get_axon_ntff_profile_hook = lambda: m._hook
    sys.modules["antenv.axon_hooks"] = m
    try:
        import antenv

        antenv.axon_hooks = m
        from trn_agent_boot.trn_boot import _ntff_profile_via_ctypes

        m._hook = _ntff_profile_via_ctypes("/opt/axon/libaxon_pjrt.so")
    except Exception:
        pass


def _patch_ldw_opt():
    import concourse.bass_utils as bass_utils

    if getattr(bass_utils, "_ldw_opt_patched", False):
        return
    orig = bass_utils.run_command

    def patched(argv, **kw):
        argv = [
            "--enable-ldw-opt=true" if a == "--enable-ldw-opt=false" else a
            for a in argv
        ]
        return orig(argv, **kw)

    bass_utils.run_command = patched
    bass_utils._ldw_opt_patched = True


def kernel(x):
    import concourse.bass_utils as bass_utils

    if LDW_OPT:
        _patch_ldw_opt()
    x = np.asarray(x, dtype=np.float32)
    assert x.shape == (B, C, H, W), x.shape
    _, _, wpack = _consts()
    nc = _get_nc()

    in_maps = [{"x": np.ascontiguousarray(x[b]), "w": wpack} for b in range(B)]
    kwargs = {}
    if TRACE:
        _install_trace_hook()
        bass_utils.upload_artifacts = lambda tmpdir: "local://" + tmpdir
        kwargs["trace"] = True
    res = bass_utils.run_bass_kernel_spmd(
        nc, in_maps, core_ids=list(range(B)), **kwargs
    )
    LAST_RESULTS["exec_time_ns"] = res.exec_time_ns
    LAST_RESULTS["mean_exec_time_ns"] = res.mean_exec_time_ns
    out = np.stack([res.results[b]["y"] for b in range(B)], axis=0)
    return out

